# revision 19
# baseline (speedup 1.0000x reference)
"""Trainium2 Bass kernel for the snake-ordered lattice GRU wavefunction model.

Strategy (data-parallel over batch, 8 cores x 128 samples):
  - 64 strictly sequential lattice sites; per site st=[hx|hy] [128,512] times a
    combined weight, with the one-hot neighbor selection reparametrized as
        sum_i sx_i W[i] = W[0] + sx_1 (W[1]-W[0]) + sx_2 (W[2]-W[0])
    so each site needs only:
      * base chunk   st @ [W1base|W2base]            -> [128,512] (ungated)
      * 4 gated chunks st @ [D1|D2]                  -> [128,512] each
      * chunk7 = [Wmerge | head]                     -> merge + logits(t-1)
  - lhsT = transposed hiddens in an SBUF ring (features on partitions);
    rhs = weights streaming; outputs land batch-on-partitions in PSUM.
  - Gating: acc = (base + bias12) then 4 per-partition-scalar MACs on VectorE
    (scalar_tensor_tensor), scalars = one-hot components from the host.
  - tanh/sigmoid on ScalarE; GRU combine h = m + u*(h~ - m) via two VectorE
    ops per half plus a transpose-accumulate on TensorE:
        hT = T(u*(h~ - m_s)) + T(m_s)   (PSUM accumulation)
  - Head (h @ [Wl1|Wl2]) folded into chunk7 of the NEXT site's GEMM.
  - Boundary sites contract filler chunks against a zero tile to keep the
    tensor engine busy (HAM warm) through the serial recurrence tail.
  - Softmax/sector-mask/log accumulation runs on host (O(B*64*3)).
"""
import os
import sys
import numpy as np

sys.path.insert(0, '/opt/trn_rl_repo')

B, NX, NY, I, H = 1024, 8, 8, 3, 256
N_TARGET, SZ = 48, 0
NCORES = 8
BC = B // NCORES          # 128 samples per core
NSITES = NX * NY          # 64
RING = 16                 # h ring buffer depth (max hy lookback is 15)

MM_DTYPE = os.environ.get("BASS_MM_DTYPE", "f32r")  # "f32" | "f32r"
NFILL = int(os.environ.get("BASS_NFILL", "4"))

_cached = {}


def _snake_sites():
    sites = []
    for ny in range(NY):
        xs = range(NX) if ny % 2 == 0 else range(NX - 1, -1, -1)
        dx = -1 if ny % 2 == 0 else 1
        for nx in xs:
            sites.append((nx, ny, nx + dx))
    return sites


SITES = _snake_sites()


def _build_program():
    import concourse.tile as tile
    from concourse import bacc, mybir

    f32 = mybir.dt.float32
    f32r = mybir.dt.float32r
    Alu = mybir.AluOpType
    Act = mybir.ActivationFunctionType

    mmdt = {"f32": f32, "f32r": f32r,
            "bf16": mybir.dt.bfloat16}[MM_DTYPE]

    nc = bacc.Bacc("TRN2", target_bir_lowering=False, debug=False,
                   num_devices=NCORES)

    wg_d = nc.dram_tensor("wg", [512, 2048], f32, kind="ExternalInput").ap()
    wbf_d = nc.dram_tensor("wbf", [512, 512], f32, kind="ExternalInput").ap()
    wbx_d = nc.dram_tensor("wbx", [512, 512], f32, kind="ExternalInput").ap()
    wby_d = nc.dram_tensor("wby", [512, 512], f32, kind="ExternalInput").ap()
    wc7a_d = nc.dram_tensor("wc7a", [512, 262], f32, kind="ExternalInput").ap()
    wc7b_d = nc.dram_tensor("wc7b", [512, 262], f32, kind="ExternalInput").ap()
    bias_d = nc.dram_tensor("bias12", [128, 512], f32, kind="ExternalInput").ap()
    ident_d = nc.dram_tensor("ident", [128, 128], f32, kind="ExternalInput").ap()
    sxy_d = nc.dram_tensor("sxy", [128, NSITES * 4], f32, kind="ExternalInput").ap()
    logits_d = nc.dram_tensor("logits", [128, NSITES * 6], f32,
                              kind="ExternalOutput").ap()

    with tile.TileContext(nc) as tc:
        with (
            tc.tile_pool(name="const", bufs=1) as constp,
            tc.tile_pool(name="work", bufs=3) as workp,
            tc.tile_pool(name="psc", bufs=1, space="PSUM") as pscp,
        ):
            # ---- persistent SBUF tiles ----
            wg_sb = [constp.tile([128, 2048], mmdt, tag=f"wg{k}", name=f"wg{k}")
                     for k in range(4)]
            wbf_sb = [constp.tile([128, 512], mmdt, tag=f"wbf{k}", name=f"wbf{k}")
                      for k in range(4)]
            wbx_sb = [constp.tile([128, 512], mmdt, tag=f"wbx{k}", name=f"wbx{k}")
                      for k in range(4)]
            wby_sb = [constp.tile([128, 512], mmdt, tag=f"wby{k}", name=f"wby{k}")
                      for k in range(4)]
            wc7a_sb = [constp.tile([128, 262], mmdt, tag=f"wa{k}", name=f"wa{k}")
                       for k in range(4)]
            wc7b_sb = [constp.tile([128, 262], mmdt, tag=f"wb{k}", name=f"wb{k}")
                       for k in range(4)]
            bias_sb = constp.tile([128, 512], f32, tag="bias")
            ident_sb = constp.tile([128, 128], f32, tag="ident")
            sxy_sb = constp.tile([128, NSITES * 4], f32, tag="sxy")
            zero_sb = constp.tile([128, 128], mmdt, tag="zero")
            ring_sb = constp.tile([128, RING * 256], mmdt, tag="ring")
            logit_sb = constp.tile([128, NSITES * 6], f32, tag="lstage")

            # ---- persistent PSUM tiles (8 banks exactly) ----
            g_ps = [pscp.tile([128, 512], f32, tag=f"g{c}", name=f"gps{c}")
                    for c in range(4)]
            cb_ps = pscp.tile([128, 512], f32, tag="cb")
            c7_ps = [pscp.tile([128, 262], f32, tag=f"c7{i}", name=f"c7ps{i}")
                     for i in range(2)]
            tr_ps = pscp.tile([128, 256], f32, tag="tr")

            # ---- loads (via f32 staging + cast when mmdt != f32) ----
            def load_weight(dst, src_ap, k):
                if MM_DTYPE == "f32":
                    nc.sync.dma_start(dst[:], src_ap)
                else:
                    stg = workp.tile(list(dst.shape), f32, tag="wstage",
                                     name=f"wstage{k}", bufs=2)
                    nc.sync.dma_start(stg[:], src_ap)
                    nc.vector.tensor_copy(dst[:], stg[:])
            # k0/k1 first: row 0 only contracts hx (k-tiles 0,1)
            for k in (0, 1):
                rows = slice(128 * k, 128 * (k + 1))
                load_weight(wby_sb[k], wby_d[rows, :], 12 + k)
                load_weight(wg_sb[k], wg_d[rows, :], k)
                load_weight(wc7a_sb[k], wc7a_d[rows, :], 16 + k)
                load_weight(wbx_sb[k], wbx_d[rows, :], 8 + k)
            for k in (2, 3):
                rows = slice(128 * k, 128 * (k + 1))
                load_weight(wg_sb[k], wg_d[rows, :], k)
                load_weight(wby_sb[k], wby_d[rows, :], 12 + k)
                load_weight(wc7a_sb[k], wc7a_d[rows, :], 16 + k)
                load_weight(wbx_sb[k], wbx_d[rows, :], 8 + k)
            for k in range(4):
                rows = slice(128 * k, 128 * (k + 1))
                load_weight(wbf_sb[k], wbf_d[rows, :], 4 + k)
                load_weight(wc7b_sb[k], wc7b_d[rows, :], 20 + k)
            nc.sync.dma_start(bias_sb[:], bias_d[:])
            nc.sync.dma_start(ident_sb[:], ident_d[:])
            nc.sync.dma_start(sxy_sb[:], sxy_d[:])
            if MM_DTYPE == "f32":
                nc.vector.memset(zero_sb[:], 0.0)
            else:
                zstg = workp.tile([128, 128], f32, tag="zstg", bufs=1)
                nc.vector.memset(zstg[:], 0.0)
                nc.vector.tensor_copy(zero_sb[:], zstg[:])

            def ring_k(site, k):
                base = (site % RING) * 256 + 128 * k
                return ring_sb[:, base:base + 128]

            du_tiles = {}
            ms_tiles = {}

            def emit_t_ms(site, half):
                sl = slice(128 * half, 128 * (half + 1))
                nc.tensor.matmul(tr_ps[:, sl], ms_tiles[site][:, sl], ident_sb[:],
                                 is_transpose=True, start=True, stop=False)

            def emit_t_du(site, half):
                sl = slice(128 * half, 128 * (half + 1))
                nc.tensor.matmul(tr_ps[:, sl], du_tiles[site][:, sl], ident_sb[:],
                                 is_transpose=True, start=False, stop=True)
                nc.scalar.copy(ring_k(site, half), tr_ps[:, sl])

            def emit_transposes(site, ms0_done=False):
                if not ms0_done:
                    emit_t_ms(site, 0)
                emit_t_du(site, 0)
                emit_t_ms(site, 1)
                emit_t_du(site, 1)

            def chunk_out_rhs(t, w7_sb, wb_sb):
                """(out_psum, rhs_tiles[k]) per chunk, in emission order."""
                lst = [(cb_ps[:], [wb_sb[k][:] for k in range(4)])]
                lst += [(g_ps[c][:], [wg_sb[k][:, 512 * c:512 * (c + 1)]
                                      for k in range(4)]) for c in range(4)]
                lst += [(c7_ps[t % 2][:], [w7_sb[k][:] for k in range(4)])]
                return lst

            def emit_phase(chunks, kts, first, last):
                for out, rhs in chunks:
                    for j, (k, lhsT) in enumerate(kts):
                        nc.tensor.matmul(out, lhsT, rhs[k],
                                         start=(first and j == 0),
                                         stop=(last and j == len(kts) - 1))

            for t, (nx, ny, nxn) in enumerate(SITES):
                x_active = (t % 8 != 0)
                y_active = (t >= 8)
                t_above = 8 * ny - 1 - (t % 8) if y_active else -1
                w7_sb = wc7a_sb if (x_active or t == 0) else wc7b_sb
                wb_sb = (wbf_sb if (x_active and y_active) else
                         (wbx_sb if x_active else wby_sb))
                chunks = chunk_out_rhs(t, w7_sb, wb_sb)

                y_kt = [(2, ring_k(t_above, 0)), (3, ring_k(t_above, 1))]
                x_kt = [(0, ring_k(t - 1, 0)), (1, ring_k(t - 1, 1))]
                z_kt = [(0, zero_sb[:]), (1, zero_sb[:])]  # exact-zero filler

                if t == 0:
                    for out, rhs in chunks:
                        nc.tensor.matmul(out, zero_sb[:], rhs[0],
                                         start=True, stop=True)
                elif not y_active:
                    emit_phase(chunks[:NFILL], z_kt, first=True, last=False)
                    emit_transposes(t - 1)
                    emit_phase(chunks[:NFILL], x_kt, first=False, last=True)
                    emit_phase(chunks[NFILL:], x_kt, first=True, last=True)
                elif not x_active:
                    emit_phase(chunks[:NFILL], z_kt, first=True, last=False)
                    emit_transposes(t - 1)
                    emit_phase(chunks[:NFILL], y_kt, first=False, last=True)
                    emit_phase(chunks[NFILL:], y_kt, first=True, last=True)
                else:
                    emit_phase(chunks[:6], y_kt, first=True, last=False)
                    emit_t_ms(t - 1, 0)
                    emit_phase(chunks[:2], z_kt, first=False, last=False)
                    emit_transposes(t - 1, ms0_done=True)
                    emit_phase(chunks[6:], y_kt, first=True, last=False)
                    emit_phase(chunks, x_kt, first=False, last=True)

                # ---- gating: acc = (base + bias12) + sum_active s_i * G_i ----
                acc = workp.tile([128, 512], f32, tag="acc")
                nc.vector.tensor_tensor(acc[:], cb_ps[:], bias_sb[:], Alu.add)
                gates = []
                if x_active:
                    gates += [(4 * t + 0, g_ps[0]), (4 * t + 1, g_ps[1])]
                if y_active:
                    gates += [(4 * t + 2, g_ps[2]), (4 * t + 3, g_ps[3])]
                for col, gps in gates:
                    nc.vector.scalar_tensor_tensor(
                        acc[:], gps[:], sxy_sb[:, col:col + 1], acc[:],
                        Alu.mult, Alu.add)

                # ---- nonlinearity + combine (halves): du = u * (h~ - ms) ----
                ms = workp.tile([128, 256], f32, tag="ms")
                nc.scalar.copy(ms[:], c7_ps[t % 2][:, 0:256])
                ht = workp.tile([128, 256], f32, tag="ht")
                u = workp.tile([128, 256], f32, tag="u")
                du = workp.tile([128, 256], f32, tag="du")
                ms_tiles[t] = ms
                du_tiles[t] = du
                for hf in (0, 1):
                    sl = slice(128 * hf, 128 * (hf + 1))
                    nc.scalar.activation(ht[:, sl], acc[:, sl], Act.Tanh)
                    nc.scalar.activation(u[:, sl],
                                         acc[:, 256 + 128 * hf:256 + 128 * (hf + 1)],
                                         Act.Sigmoid)
                    nc.vector.tensor_tensor(du[:, sl], ht[:, sl], ms[:, sl],
                                            Alu.subtract)
                    nc.vector.tensor_tensor(du[:, sl], du[:, sl], u[:, sl],
                                            Alu.mult)

                # logits(t-1) out of chunk7
                if t > 0:
                    nc.scalar.copy(logit_sb[:, 6 * (t - 1):6 * t],
                                   c7_ps[t % 2][:, 256:262])

            # ---- tail: head for site 63 ----
            emit_transposes(NSITES - 1)
            nc.tensor.matmul(c7_ps[0][:], ring_k(NSITES - 1, 0),
                             wc7a_sb[0][:], start=True, stop=False)
            nc.tensor.matmul(c7_ps[0][:], ring_k(NSITES - 1, 1),
                             wc7a_sb[1][:], start=False, stop=True)
            nc.scalar.copy(logit_sb[:, 6 * (NSITES - 1):6 * NSITES],
                           c7_ps[0][:, 256:262])

            nc.sync.dma_start(logits_d[:], logit_sb[:])

    nc.compile()
    return nc


def _host_pre(samples, W1, W2, Wmerge, Wl1, Wl2, b1, b2):
    oh = np.zeros((B, NX, NY, I), np.float32)
    idx = np.indices(samples.shape)
    oh[idx[0], idx[1], idx[2], samples] = 1.0
    SX = np.zeros((NSITES, B, I), np.float32)
    SY = np.zeros((NSITES, B, I), np.float32)
    for t, (nx, ny, nxn) in enumerate(SITES):
        if 0 <= nxn < NX:
            SX[t] = oh[:, nxn, ny]
        if ny > 0:
            SY[t] = oh[:, nx, ny - 1]

    def DD(i, b):
        return np.concatenate([W1[i] - W1[b], W2[i] - W2[b]], axis=1)

    wg = np.concatenate([DD(1, 0), DD(2, 0), DD(4, 3), DD(5, 3)], axis=1)
    wbf = np.concatenate([W1[0] + W1[3], W2[0] + W2[3]], axis=1)
    wbx = np.concatenate([W1[0], W2[0]], axis=1)
    wby = np.concatenate([W1[3], W2[3]], axis=1)
    Wl = np.concatenate([Wl1, Wl2], axis=1)
    z = np.zeros((H, 6), np.float32)
    wc7a = np.concatenate([Wmerge, np.concatenate([Wl, z], 0)], axis=1)
    wc7b = np.concatenate([Wmerge, np.concatenate([z, Wl], 0)], axis=1)
    bias12 = np.broadcast_to(
        np.concatenate([b1, b2]).astype(np.float32), (128, 512)).copy()
    c = np.ascontiguousarray
    return SX, SY, c(wg), c(wbf), c(wbx), c(wby), c(wc7a), c(wc7b), bias12


def _host_post(samples, logits, bl1, bl2):
    """logits: [B, NSITES, 6].  Returns (0.5*log_a, log_p)."""
    log_a = np.zeros(B, np.float32)
    log_p = np.zeros(B, np.float32)
    bl_up = (N_TARGET + 2 * SZ) // 2
    bl_dn = (N_TARGET - 2 * SZ) // 2
    bl_hole = NX * NY - N_TARGET
    n_up = np.zeros(B, np.float32)
    n_dn = np.zeros(B, np.float32)
    ar = np.arange(B)
    for t, (nx, ny, nxn) in enumerate(SITES):
        l1 = logits[:, t, 0:3] + bl1
        l2 = logits[:, t, 3:6] + bl2
        e = np.exp(l1 - l1.max(axis=1, keepdims=True))
        probs = e / e.sum(axis=1, keepdims=True)
        phase = np.float32(np.pi) * (l2 / (1.0 + np.abs(l2)))
        m_up = (bl_up - n_up > 0).astype(np.float32)
        m_dn = (bl_dn - n_dn > 0).astype(np.float32)
        m_hole = (bl_hole - (t - n_up - n_dn) > 0).astype(np.float32)
        mask = np.stack([m_hole, m_dn, m_up], axis=1)
        amp = probs * mask
        amp = amp / np.maximum(amp.sum(axis=1, keepdims=True), 1e-30)
        s = samples[:, nx, ny]
        log_a += np.log(np.clip(amp[ar, s], 1e-12, None)).astype(np.float32)
        log_p += phase[ar, s].astype(np.float32)
        n_up += (s == 2)
        n_dn += (s == 1)
    return (0.5 * log_a).astype(np.float32), log_p.astype(np.float32)


last_results = None  # exposed for test.py profiling


def _install_neff_saver(dst_dir):
    """Monkeypatch bass2jax's BIR->NEFF compile to retain a NEFF copy for
    neuron-profile (the axon path normally discards it)."""
    import shutil
    from concourse import bass2jax as b2j
    if getattr(b2j, "_neff_saver_installed", False):
        return
    orig = b2j.compile_bir_kernel

    def wrapper(bir_json, tmpdir, neff_name="file.neff", **kw):
        out = orig(bir_json, tmpdir, neff_name=neff_name, **kw)
        try:
            shutil.copy(out, os.path.join(dst_dir, "kernel.neff"))
        except Exception:
            pass
        return out

    b2j.compile_bir_kernel = wrapper
    b2j._neff_saver_installed = True


def kernel(samples, W1, b1, W2, b2, Wmerge, Wl1, bl1, Wl2, bl2):
    global last_results
    from concourse.bass_utils import run_bass_kernel_spmd

    samples = np.asarray(samples).astype(np.int64)
    f = lambda x: np.asarray(x, dtype=np.float32)
    W1, b1, W2, b2 = f(W1), f(b1), f(W2), f(b2)
    Wmerge, Wl1, bl1, Wl2, bl2 = f(Wmerge), f(Wl1), f(bl1), f(Wl2), f(bl2)

    SX, SY, wg, wbf, wbx, wby, wc7a, wc7b, bias12 = _host_pre(
        samples, W1, W2, Wmerge, Wl1, Wl2, b1, b2)

    if "nc" not in _cached:
        _cached["nc"] = _build_program()
    nc = _cached["nc"]

    ident = np.eye(128, dtype=np.float32)
    core_ids = list(range(NCORES))
    in_maps = []
    for c in core_ids:
        sl = slice(c * BC, (c + 1) * BC)
        sxy = np.empty((BC, NSITES * 4), np.float32)
        for t in range(NSITES):
            sxy[:, 4 * t + 0] = SX[t, sl, 1]
            sxy[:, 4 * t + 1] = SX[t, sl, 2]
            sxy[:, 4 * t + 2] = SY[t, sl, 1]
            sxy[:, 4 * t + 3] = SY[t, sl, 2]
        in_maps.append({"wg": wg, "wbf": wbf, "wbx": wbx, "wby": wby,
                        "wc7a": wc7a, "wc7b": wc7b,
                        "bias12": bias12, "ident": ident, "sxy": sxy})

    ntff_dir = os.environ.get("BASS_NTFF_DIR", "")
    if ntff_dir:
        os.makedirs(ntff_dir, exist_ok=True)
        _install_neff_saver(ntff_dir)
        from trn_agent_boot.trn_boot import _ntff_profile_via_ctypes
        hook = _ntff_profile_via_ctypes("/opt/axon/libaxon_pjrt.so")
        with hook(ntff_dir, None):
            res = run_bass_kernel_spmd(nc, in_maps, core_ids)
    else:
        res = run_bass_kernel_spmd(nc, in_maps, core_ids)
    last_results = res

    logits = np.concatenate(
        [res.results[c]["logits"].reshape(BC, NSITES, 6) for c in core_ids],
        axis=0)
    return _host_post(samples, logits, bl1, bl2)


# revision 20
# speedup vs baseline: 1.0165x; 1.0165x over previous
"""Trainium2 Bass kernel for the snake-ordered lattice GRU wavefunction model.

Strategy (data-parallel over batch, 8 cores x 128 samples):
  - 64 strictly sequential lattice sites; per site st=[hx|hy] [128,512] times a
    combined weight, with the one-hot neighbor selection reparametrized as
        sum_i sx_i W[i] = W[0] + sx_1 (W[1]-W[0]) + sx_2 (W[2]-W[0])
    so each site needs only:
      * base chunk   st @ [W1base|W2base]            -> [128,512] (ungated)
      * 4 gated chunks st @ [D1|D2]                  -> [128,512] each
      * chunk7 = [Wmerge | head]                     -> merge + logits(t-1)
  - lhsT = transposed hiddens in an SBUF ring (features on partitions);
    rhs = weights streaming; outputs land batch-on-partitions in PSUM.
  - Gating: acc = (base + bias12) then 4 per-partition-scalar MACs on VectorE
    (scalar_tensor_tensor), scalars = one-hot components from the host.
  - tanh/sigmoid on ScalarE; GRU combine h = m + u*(h~ - m) via two VectorE
    ops per half plus a transpose-accumulate on TensorE:
        hT = T(u*(h~ - m_s)) + T(m_s)   (PSUM accumulation)
  - Head (h @ [Wl1|Wl2]) folded into chunk7 of the NEXT site's GEMM.
  - Boundary sites contract filler chunks against a zero tile to keep the
    tensor engine busy (HAM warm) through the serial recurrence tail.
  - Softmax/sector-mask/log accumulation runs on host (O(B*64*3)).
"""
import os
import sys
import numpy as np

sys.path.insert(0, '/opt/trn_rl_repo')

B, NX, NY, I, H = 1024, 8, 8, 3, 256
N_TARGET, SZ = 48, 0
NCORES = 8
BC = B // NCORES          # 128 samples per core
NSITES = NX * NY          # 64
RING = 16                 # h ring buffer depth (max hy lookback is 15)

MM_DTYPE = os.environ.get("BASS_MM_DTYPE", "f32r")  # "f32" | "f32r"
NFILL = int(os.environ.get("BASS_NFILL", "4"))

_cached = {}


def _snake_sites():
    sites = []
    for ny in range(NY):
        xs = range(NX) if ny % 2 == 0 else range(NX - 1, -1, -1)
        dx = -1 if ny % 2 == 0 else 1
        for nx in xs:
            sites.append((nx, ny, nx + dx))
    return sites


SITES = _snake_sites()


def _build_program():
    import concourse.tile as tile
    from concourse import bacc, mybir

    f32 = mybir.dt.float32
    f32r = mybir.dt.float32r
    Alu = mybir.AluOpType
    Act = mybir.ActivationFunctionType

    mmdt = {"f32": f32, "f32r": f32r,
            "bf16": mybir.dt.bfloat16}[MM_DTYPE]

    nc = bacc.Bacc("TRN2", target_bir_lowering=False, debug=False,
                   num_devices=NCORES)

    wg_d = nc.dram_tensor("wg", [512, 2048], f32, kind="ExternalInput").ap()
    wbf_d = nc.dram_tensor("wbf", [512, 512], f32, kind="ExternalInput").ap()
    wbx_d = nc.dram_tensor("wbx", [512, 512], f32, kind="ExternalInput").ap()
    wby_d = nc.dram_tensor("wby", [512, 512], f32, kind="ExternalInput").ap()
    wc7a_d = nc.dram_tensor("wc7a", [512, 262], f32, kind="ExternalInput").ap()
    wc7b_d = nc.dram_tensor("wc7b", [512, 262], f32, kind="ExternalInput").ap()
    bias_d = nc.dram_tensor("bias12", [128, 512], f32, kind="ExternalInput").ap()
    ident_d = nc.dram_tensor("ident", [128, 128], f32, kind="ExternalInput").ap()
    sxy_d = nc.dram_tensor("sxy", [128, NSITES * 4], f32, kind="ExternalInput").ap()
    logits_d = nc.dram_tensor("logits", [128, NSITES * 6], f32,
                              kind="ExternalOutput").ap()

    with tile.TileContext(nc) as tc:
        with (
            tc.tile_pool(name="const", bufs=1) as constp,
            tc.tile_pool(name="work", bufs=3) as workp,
            tc.tile_pool(name="psc", bufs=1, space="PSUM") as pscp,
        ):
            # ---- persistent SBUF tiles ----
            wg_sb = [constp.tile([128, 2048], mmdt, tag=f"wg{k}", name=f"wg{k}")
                     for k in range(4)]
            wbf_sb = [constp.tile([128, 512], mmdt, tag=f"wbf{k}", name=f"wbf{k}")
                      for k in range(4)]
            wbx_sb = [constp.tile([128, 512], mmdt, tag=f"wbx{k}", name=f"wbx{k}")
                      for k in range(4)]
            wby_sb = [constp.tile([128, 512], mmdt, tag=f"wby{k}", name=f"wby{k}")
                      for k in range(4)]
            wc7a_sb = [constp.tile([128, 262], mmdt, tag=f"wa{k}", name=f"wa{k}")
                       for k in range(4)]
            wc7b_sb = [constp.tile([128, 262], mmdt, tag=f"wb{k}", name=f"wb{k}")
                       for k in range(4)]
            bias_sb = constp.tile([128, 512], f32, tag="bias")
            ident_sb = constp.tile([128, 128], f32, tag="ident")
            sxy_sb = constp.tile([128, NSITES * 4], f32, tag="sxy")
            zero_sb = constp.tile([128, 128], mmdt, tag="zero")
            ring_sb = constp.tile([128, RING * 256], mmdt, tag="ring")
            logit_sb = constp.tile([128, NSITES * 6], f32, tag="lstage")

            # ---- persistent PSUM tiles (8 banks exactly) ----
            g_ps = [pscp.tile([128, 512], f32, tag=f"g{c}", name=f"gps{c}")
                    for c in range(4)]
            cb_ps = pscp.tile([128, 512], f32, tag="cb")
            c7_ps = [pscp.tile([128, 262], f32, tag=f"c7{i}", name=f"c7ps{i}")
                     for i in range(2)]
            tr_ps = pscp.tile([128, 256], f32, tag="tr")

            # ---- loads (via f32 staging + cast when mmdt != f32) ----
            def load_weight(dst, src_ap, k):
                if MM_DTYPE == "f32":
                    nc.sync.dma_start(dst[:], src_ap)
                else:
                    stg = workp.tile(list(dst.shape), f32, tag="wstage",
                                     name=f"wstage{k}", bufs=2)
                    nc.sync.dma_start(stg[:], src_ap)
                    nc.vector.tensor_copy(dst[:], stg[:])
            # k0/k1 first: row 0 only contracts hx (k-tiles 0,1)
            for k in (0, 1):
                rows = slice(128 * k, 128 * (k + 1))
                load_weight(wby_sb[k], wby_d[rows, :], 12 + k)
                load_weight(wg_sb[k], wg_d[rows, :], k)
                load_weight(wc7a_sb[k], wc7a_d[rows, :], 16 + k)
                load_weight(wbx_sb[k], wbx_d[rows, :], 8 + k)
            for k in (2, 3):
                rows = slice(128 * k, 128 * (k + 1))
                load_weight(wg_sb[k], wg_d[rows, :], k)
                load_weight(wby_sb[k], wby_d[rows, :], 12 + k)
                load_weight(wc7a_sb[k], wc7a_d[rows, :], 16 + k)
                load_weight(wbx_sb[k], wbx_d[rows, :], 8 + k)
            for k in range(4):
                rows = slice(128 * k, 128 * (k + 1))
                load_weight(wbf_sb[k], wbf_d[rows, :], 4 + k)
                load_weight(wc7b_sb[k], wc7b_d[rows, :], 20 + k)
            nc.sync.dma_start(bias_sb[:], bias_d[:])
            nc.sync.dma_start(ident_sb[:], ident_d[:])
            nc.sync.dma_start(sxy_sb[:], sxy_d[:])
            if MM_DTYPE == "f32":
                nc.vector.memset(zero_sb[:], 0.0)
            else:
                zstg = workp.tile([128, 128], f32, tag="zstg", bufs=1)
                nc.vector.memset(zstg[:], 0.0)
                nc.vector.tensor_copy(zero_sb[:], zstg[:])

            def ring_k(site, k):
                base = (site % RING) * 256 + 128 * k
                return ring_sb[:, base:base + 128]

            du_tiles = {}
            ms_tiles = {}

            def emit_t_ms(site, half):
                sl = slice(128 * half, 128 * (half + 1))
                nc.tensor.matmul(tr_ps[:, sl], ms_tiles[site][:, sl], ident_sb[:],
                                 is_transpose=True, start=True, stop=False)

            def emit_t_du(site, half):
                sl = slice(128 * half, 128 * (half + 1))
                nc.tensor.matmul(tr_ps[:, sl], du_tiles[site][:, sl], ident_sb[:],
                                 is_transpose=True, start=False, stop=True)
                nc.scalar.copy(ring_k(site, half), tr_ps[:, sl])

            def emit_transposes(site, ms0_done=False):
                if not ms0_done:
                    emit_t_ms(site, 0)
                emit_t_du(site, 0)
                emit_t_ms(site, 1)
                emit_t_du(site, 1)

            def chunk_out_rhs(t, w7_sb, wb_sb):
                """(out_psum, rhs_tiles[k]) per chunk, in emission order."""
                lst = [(cb_ps[:], [wb_sb[k][:] for k in range(4)])]
                lst += [(g_ps[c][:], [wg_sb[k][:, 512 * c:512 * (c + 1)]
                                      for k in range(4)]) for c in range(4)]
                lst += [(c7_ps[t % 2][:], [w7_sb[k][:] for k in range(4)])]
                return lst

            def emit_phase(chunks, kts, first, last):
                for out, rhs in chunks:
                    for j, (k, lhsT) in enumerate(kts):
                        nc.tensor.matmul(out, lhsT, rhs[k],
                                         start=(first and j == 0),
                                         stop=(last and j == len(kts) - 1))

            for t, (nx, ny, nxn) in enumerate(SITES):
                x_active = (t % 8 != 0)
                y_active = (t >= 8)
                t_above = 8 * ny - 1 - (t % 8) if y_active else -1
                w7_sb = wc7a_sb if (x_active or t == 0) else wc7b_sb
                wb_sb = (wbf_sb if (x_active and y_active) else
                         (wbx_sb if x_active else wby_sb))
                chunks = chunk_out_rhs(t, w7_sb, wb_sb)

                y_kt = [(2, ring_k(t_above, 0)), (3, ring_k(t_above, 1))]
                x_kt = [(0, ring_k(t - 1, 0)), (1, ring_k(t - 1, 1))]
                z_kt = [(0, zero_sb[:]), (1, zero_sb[:])]  # exact-zero filler

                if t == 0:
                    for out, rhs in chunks:
                        nc.tensor.matmul(out, zero_sb[:], rhs[0],
                                         start=True, stop=True)
                elif not y_active:
                    emit_phase(chunks[:NFILL], z_kt, first=True, last=False)
                    emit_transposes(t - 1)
                    emit_phase(chunks[:NFILL], x_kt, first=False, last=True)
                    emit_phase(chunks[NFILL:], x_kt, first=True, last=True)
                elif not x_active:
                    emit_phase(chunks[:NFILL], z_kt, first=True, last=False)
                    emit_transposes(t - 1)
                    emit_phase(chunks[:NFILL], y_kt, first=False, last=True)
                    emit_phase(chunks[NFILL:], y_kt, first=True, last=True)
                else:
                    emit_phase(chunks[:6], y_kt, first=True, last=False)
                    emit_phase(chunks[:1], z_kt, first=False, last=False)
                    emit_transposes(t - 1)
                    emit_phase(chunks[6:], y_kt, first=True, last=False)
                    emit_phase(chunks, x_kt, first=False, last=True)

                # ---- gating: acc = (base + bias12) + sum_active s_i * G_i ----
                acc = workp.tile([128, 512], f32, tag="acc")
                nc.vector.tensor_tensor(acc[:], cb_ps[:], bias_sb[:], Alu.add)
                gates = []
                if x_active:
                    gates += [(4 * t + 0, g_ps[0]), (4 * t + 1, g_ps[1])]
                if y_active:
                    gates += [(4 * t + 2, g_ps[2]), (4 * t + 3, g_ps[3])]
                for col, gps in gates:
                    nc.vector.scalar_tensor_tensor(
                        acc[:], gps[:], sxy_sb[:, col:col + 1], acc[:],
                        Alu.mult, Alu.add)

                # ---- nonlinearity + combine (halves): du = u * (h~ - ms) ----
                ms = workp.tile([128, 256], f32, tag="ms")
                nc.scalar.copy(ms[:], c7_ps[t % 2][:, 0:256])
                ht = workp.tile([128, 256], f32, tag="ht")
                u = workp.tile([128, 256], f32, tag="u")
                du = workp.tile([128, 256], f32, tag="du")
                ms_tiles[t] = ms
                du_tiles[t] = du
                for hf in (0, 1):
                    sl = slice(128 * hf, 128 * (hf + 1))
                    nc.scalar.activation(ht[:, sl], acc[:, sl], Act.Tanh)
                    nc.scalar.activation(u[:, sl],
                                         acc[:, 256 + 128 * hf:256 + 128 * (hf + 1)],
                                         Act.Sigmoid)
                    nc.vector.tensor_tensor(du[:, sl], ht[:, sl], ms[:, sl],
                                            Alu.subtract)
                    nc.vector.tensor_tensor(du[:, sl], du[:, sl], u[:, sl],
                                            Alu.mult)

                # logits(t-1) out of chunk7
                if t > 0:
                    nc.scalar.copy(logit_sb[:, 6 * (t - 1):6 * t],
                                   c7_ps[t % 2][:, 256:262])

            # ---- tail: head for site 63 ----
            emit_transposes(NSITES - 1)
            nc.tensor.matmul(c7_ps[0][:], ring_k(NSITES - 1, 0),
                             wc7a_sb[0][:], start=True, stop=False)
            nc.tensor.matmul(c7_ps[0][:], ring_k(NSITES - 1, 1),
                             wc7a_sb[1][:], start=False, stop=True)
            nc.scalar.copy(logit_sb[:, 6 * (NSITES - 1):6 * NSITES],
                           c7_ps[0][:, 256:262])

            nc.sync.dma_start(logits_d[:], logit_sb[:])

    nc.compile()
    return nc


def _host_pre(samples, W1, W2, Wmerge, Wl1, Wl2, b1, b2):
    oh = np.zeros((B, NX, NY, I), np.float32)
    idx = np.indices(samples.shape)
    oh[idx[0], idx[1], idx[2], samples] = 1.0
    SX = np.zeros((NSITES, B, I), np.float32)
    SY = np.zeros((NSITES, B, I), np.float32)
    for t, (nx, ny, nxn) in enumerate(SITES):
        if 0 <= nxn < NX:
            SX[t] = oh[:, nxn, ny]
        if ny > 0:
            SY[t] = oh[:, nx, ny - 1]

    def DD(i, b):
        return np.concatenate([W1[i] - W1[b], W2[i] - W2[b]], axis=1)

    wg = np.concatenate([DD(1, 0), DD(2, 0), DD(4, 3), DD(5, 3)], axis=1)
    wbf = np.concatenate([W1[0] + W1[3], W2[0] + W2[3]], axis=1)
    wbx = np.concatenate([W1[0], W2[0]], axis=1)
    wby = np.concatenate([W1[3], W2[3]], axis=1)
    Wl = np.concatenate([Wl1, Wl2], axis=1)
    z = np.zeros((H, 6), np.float32)
    wc7a = np.concatenate([Wmerge, np.concatenate([Wl, z], 0)], axis=1)
    wc7b = np.concatenate([Wmerge, np.concatenate([z, Wl], 0)], axis=1)
    bias12 = np.broadcast_to(
        np.concatenate([b1, b2]).astype(np.float32), (128, 512)).copy()
    c = np.ascontiguousarray
    return SX, SY, c(wg), c(wbf), c(wbx), c(wby), c(wc7a), c(wc7b), bias12


def _host_post(samples, logits, bl1, bl2):
    """logits: [B, NSITES, 6].  Returns (0.5*log_a, log_p)."""
    log_a = np.zeros(B, np.float32)
    log_p = np.zeros(B, np.float32)
    bl_up = (N_TARGET + 2 * SZ) // 2
    bl_dn = (N_TARGET - 2 * SZ) // 2
    bl_hole = NX * NY - N_TARGET
    n_up = np.zeros(B, np.float32)
    n_dn = np.zeros(B, np.float32)
    ar = np.arange(B)
    for t, (nx, ny, nxn) in enumerate(SITES):
        l1 = logits[:, t, 0:3] + bl1
        l2 = logits[:, t, 3:6] + bl2
        e = np.exp(l1 - l1.max(axis=1, keepdims=True))
        probs = e / e.sum(axis=1, keepdims=True)
        phase = np.float32(np.pi) * (l2 / (1.0 + np.abs(l2)))
        m_up = (bl_up - n_up > 0).astype(np.float32)
        m_dn = (bl_dn - n_dn > 0).astype(np.float32)
        m_hole = (bl_hole - (t - n_up - n_dn) > 0).astype(np.float32)
        mask = np.stack([m_hole, m_dn, m_up], axis=1)
        amp = probs * mask
        amp = amp / np.maximum(amp.sum(axis=1, keepdims=True), 1e-30)
        s = samples[:, nx, ny]
        log_a += np.log(np.clip(amp[ar, s], 1e-12, None)).astype(np.float32)
        log_p += phase[ar, s].astype(np.float32)
        n_up += (s == 2)
        n_dn += (s == 1)
    return (0.5 * log_a).astype(np.float32), log_p.astype(np.float32)


last_results = None  # exposed for test.py profiling


def _install_neff_saver(dst_dir):
    """Monkeypatch bass2jax's BIR->NEFF compile to retain a NEFF copy for
    neuron-profile (the axon path normally discards it)."""
    import shutil
    from concourse import bass2jax as b2j
    if getattr(b2j, "_neff_saver_installed", False):
        return
    orig = b2j.compile_bir_kernel

    def wrapper(bir_json, tmpdir, neff_name="file.neff", **kw):
        out = orig(bir_json, tmpdir, neff_name=neff_name, **kw)
        try:
            shutil.copy(out, os.path.join(dst_dir, "kernel.neff"))
        except Exception:
            pass
        return out

    b2j.compile_bir_kernel = wrapper
    b2j._neff_saver_installed = True


def kernel(samples, W1, b1, W2, b2, Wmerge, Wl1, bl1, Wl2, bl2):
    global last_results
    from concourse.bass_utils import run_bass_kernel_spmd

    samples = np.asarray(samples).astype(np.int64)
    f = lambda x: np.asarray(x, dtype=np.float32)
    W1, b1, W2, b2 = f(W1), f(b1), f(W2), f(b2)
    Wmerge, Wl1, bl1, Wl2, bl2 = f(Wmerge), f(Wl1), f(bl1), f(Wl2), f(bl2)

    SX, SY, wg, wbf, wbx, wby, wc7a, wc7b, bias12 = _host_pre(
        samples, W1, W2, Wmerge, Wl1, Wl2, b1, b2)

    if "nc" not in _cached:
        _cached["nc"] = _build_program()
    nc = _cached["nc"]

    ident = np.eye(128, dtype=np.float32)
    core_ids = list(range(NCORES))
    in_maps = []
    for c in core_ids:
        sl = slice(c * BC, (c + 1) * BC)
        sxy = np.empty((BC, NSITES * 4), np.float32)
        for t in range(NSITES):
            sxy[:, 4 * t + 0] = SX[t, sl, 1]
            sxy[:, 4 * t + 1] = SX[t, sl, 2]
            sxy[:, 4 * t + 2] = SY[t, sl, 1]
            sxy[:, 4 * t + 3] = SY[t, sl, 2]
        in_maps.append({"wg": wg, "wbf": wbf, "wbx": wbx, "wby": wby,
                        "wc7a": wc7a, "wc7b": wc7b,
                        "bias12": bias12, "ident": ident, "sxy": sxy})

    ntff_dir = os.environ.get("BASS_NTFF_DIR", "")
    if ntff_dir:
        os.makedirs(ntff_dir, exist_ok=True)
        _install_neff_saver(ntff_dir)
        from trn_agent_boot.trn_boot import _ntff_profile_via_ctypes
        hook = _ntff_profile_via_ctypes("/opt/axon/libaxon_pjrt.so")
        with hook(ntff_dir, None):
            res = run_bass_kernel_spmd(nc, in_maps, core_ids)
    else:
        res = run_bass_kernel_spmd(nc, in_maps, core_ids)
    last_results = res

    logits = np.concatenate(
        [res.results[c]["logits"].reshape(BC, NSITES, 6) for c in core_ids],
        axis=0)
    return _host_post(samples, logits, bl1, bl2)
